# revision 36
# baseline (speedup 1.0000x reference)
"""Trainium2 Bass kernel for nn_Encoder_MLP (embedding gather + sum + 2-layer MLP tail).

Reference computation:
    x = where(gate_seq < 0, A, gate_seq)            # [B, T]   (inputs never negative)
    Wr = W1.reshape(T, V, HID)
    h  = Wr[arange(T)[None,:], x].sum(axis=1) + b1  # [B, HID]  gather B*T rows, sum over T
    h  = relu(h); h = relu(h @ W2 + b2); out = h @ W3 + b3

Sharding (8 cores): data-parallel over the batch axis, W1 fully replicated
(bf16, 512 MB/core in HBM). Core m owns batches [8m, 8m+8) and gathers all
T=256 positions for them. NO collective: cross-core sync absorbs NEFF launch
skew (95+us measured on a T-sharded variant).

Gather phase: dma_gather idx are int16 (<=32768 addressable elements per
call), so a call covers 16 positions via 2-row blocks (PAIR window, 128 idx,
fetches 2x and selects by parity) or 8 positions of single rows (SINGLE
window, 64 idx). Measured: per-call Q7 fixed ~0.9-1.1us + ~6ns/idx; drain
~140 GB/s aggregate for random 512B-1KB descriptors. Phase ~= cold-start +
~2.2us per 4-call wave + drain, so the schedule uses 5 waves per queue
(3 pairs + 2 singles, pairs first so the phase ends on cheap 32 KB drains):
12 PAIR windows (positions 0..191) + 8 SINGLE windows (192..255).

Reduce (transposed fold — produces hT directly, no PE transpose after):
All fold matmuls write two PSUM groups hT_c [128, 8] (c = hid chunk):
- pair windows: wanted half of block picked by parity = x & 1:
    hT_c += d_w[:, c]^T . om_w  +  accEven[:, c]^T . bmask
  where d_w = odd_w - even_w (DVE), om_w[p,:] = bmask[p,:]*parity[p,w],
  accEven = progressive sum of all even halves (DVE adds).
- single windows: hT_c += g_u[0:64, c]^T . bmask[0:64, :].
Then relu(+b1T) from PSUM on two engines in parallel (scalar.activation for
chunk 0, DVE dual-op tensor_scalar for chunk 1) -> hTr [128,2,8], 4 matmuls
with W2 128x128 chunks as lhsT -> relu(+b2T) -> two 128-col output PSUM
groups (rank-1 b3 matmul + 2 data matmuls each; group-0 copy overlaps
group-1 matmuls) -> one [8, 256] f32 DMA. Host concatenates per-core outputs.

Index layout: idx list position i lives at idx_tile[i%16, base + i//16]
(16-partition wrap, replicated x8 for the 8 Q7 cores); i = j*8 + b so
partition p holds batch p%8. The j*2048 / j*4096 rebase is an inline const
added on device; the pair-half x>>1 is a device tensor_scalar shift; the
host only permutes/retypes gate_seq (value-independent layout marshaling).

ALL non-W1 inputs (gate permutations, ubias, w2lh, w3, b1t, b2t, bmask, b3)
ride ONE byte-packed blob DMA (fewer DMA sems to drain in the epilogue).
num_idxs registers are materialized once per distinct count (2 MOVEs, not 20).

Known dead ends (measured): hoisting the library load above the framework
barrier (-> barrier's Pool DRAIN absorbs the ~10us IRAM fetch, +11us); native
indirect_dma_start (resident, no library) — drain is HBM-latency-serial per
descriptor (~21 GB/s) and returns corrupt data for these shapes; fp8 W1
(~32x bf16 quantization error, blows the 2e-2 rel tolerance); splitting the
8 KB output DMA (+0.7us fixed per DMA); 4-wave all-pair (drain-bound) and
6/7/8-wave singles-heavy schedules (wave-bound).
"""

import sys

import numpy as np

if "/opt/trn_rl_repo" not in sys.path:
    sys.path.insert(0, "/opt/trn_rl_repo")

B = 64
T = 256
V = 4096
HID = 256
OUT = 256
NCORES = 8
BPC = B // NCORES          # batches per core = 8
NQ = 4                     # SWDGE queues

NP = 12                    # pair windows (16 positions each): positions [0, 192)
PWIN_POS = 16
PWIN_ROWS = PWIN_POS * V   # 65536 rows = 32768 2-row blocks
P_NIDX = BPC * PWIN_POS    # 128 idx per pair call
P_IDXC = P_NIDX // 16      # 8 idx cols per pair window

NS = 8                     # single windows (8 positions each): positions [192, 256)
SBASE_POS = NP * PWIN_POS  # 192
SWIN_POS = 8
SWIN_ROWS = SWIN_POS * V   # 32768 rows
S_NIDX = BPC * SWIN_POS    # 64 idx per single call
S_IDXC = S_NIDX // 16      # 4 idx cols per single window

PCOLS = NP * P_IDXC        # 64 idx columns for the pair half
SCOLS = NS * S_IDXC        # 64 idx columns for the single half
GCOLS = PCOLS + SCOLS + NP # gate_all: idx halves + gate_T

# blob byte offsets (per partition) — ONE input DMA carries everything
OFF_GATE = 0                    # [128, GCOLS] i16 (idx cols + gate_T)
OFF_UB = 2 * GCOLS              # [128, PCOLS+SCOLS] i16 ubias
OFF_W2 = OFF_UB + 2 * (PCOLS + SCOLS)  # [128, 2, 2, 128] bf16 -> 1024 B
OFF_W3 = OFF_W2 + 1024          # [128, 2, 256] bf16 -> 1024 B
OFF_B1 = OFF_W3 + 1024          # [128, 2] f32 -> 8 B
OFF_B2 = OFF_B1 + 8             # [128, 2] f32 -> 8 B
OFF_BM = OFF_B2 + 8             # [128, 8] bf16 -> 16 B
OFF_B3 = OFF_BM + 16            # [128, 256] bf16 (replicated rows) -> 512 B
BLOB_BYTES = OFF_B3 + 512
assert OFF_B1 % 4 == 0, OFF_B1

_CACHE = {}


def _host_consts():
    p = np.arange(128)[:, None]
    # pair half: i = (col % 8)*16 + p%16; j = i//8 -> rebase j*2048
    colp = np.arange(PCOLS)[None, :]
    ip = (colp % P_IDXC) * 16 + (p % 16)
    ub_p = (ip // BPC) * (V // 2)
    # single half: i = (col % 4)*16 + p%16; j = i//8 -> rebase j*4096
    cols = np.arange(SCOLS)[None, :]
    i_s = (cols % S_IDXC) * 16 + (p % 16)
    ub_s = (i_s // BPC) * V
    ubias = np.concatenate(
        [np.broadcast_to(ub_p, (128, PCOLS)), np.broadcast_to(ub_s, (128, SCOLS))],
        axis=1,
    ).astype(np.int16)
    return np.ascontiguousarray(ubias)


def _build_nc():
    import concourse.bacc as bacc
    import concourse.mybir as mybir
    import concourse.tile as tile

    f32 = mybir.dt.float32
    bf16 = mybir.dt.bfloat16
    i16 = mybir.dt.int16
    u8 = mybir.dt.uint8
    Relu = mybir.ActivationFunctionType.Relu
    add = mybir.AluOpType.add
    sub = mybir.AluOpType.subtract
    mult = mybir.AluOpType.mult
    shr = mybir.AluOpType.logical_shift_right
    band = mybir.AluOpType.bitwise_and

    nc = bacc.Bacc(
        "TRN2",
        target_bir_lowering=False,
        debug=False,
        num_devices=NCORES,
        num_swdge_queues=NQ,
    )

    w1_d = nc.dram_tensor("w1", [T * V, HID], bf16, kind="ExternalInput")
    blob_d = nc.dram_tensor("blob", [128, BLOB_BYTES], u8, kind="ExternalInput")
    out_d = nc.dram_tensor("out", [BPC, OUT], f32, kind="ExternalOutput")

    # Issue the mlp ucode library load before any Tile-scheduled work so the
    # ~10us Q7 library fetch overlaps the NEFF prologue instead of stalling
    # the first dma_gather until ~16us.
    from concourse import library_config

    mpc_inst = nc.gpsimd.load_library(library_config.mlp)

    with tile.TileContext(nc) as tc:
        with (
            tc.tile_pool(name="const", bufs=1) as const,
            tc.tile_pool(name="gat", bufs=1) as gat,
            tc.tile_pool(name="work", bufs=2) as work,
            tc.tile_pool(name="psum", bufs=1, space="PSUM") as psum,
        ):
            # ---- ONE input DMA: gate + ubias + MLP consts (bitcast views) ----
            blob = const.tile([128, BLOB_BYTES], u8, tag="blob")
            nc.sync.dma_start(blob[:], blob_d[:])
            ga = blob[:, OFF_GATE : OFF_GATE + 2 * GCOLS].bitcast(i16)  # [128, 136]
            ub = blob[:, OFF_UB : OFF_UB + 256].bitcast(i16)            # [128, 128]

            # ---- critical path: indices (pair half: x>>1 + j*2048; single: x + j*4096)
            idx = const.tile([128, PCOLS + SCOLS], i16, tag="idx")
            nc.vector.tensor_scalar(idx[:, 0:PCOLS], ga[:, 0:PCOLS], 1, None, shr)
            nc.vector.tensor_tensor(idx[:, 0:PCOLS], idx[:, 0:PCOLS], ub[:, 0:PCOLS], add)
            nc.vector.tensor_tensor(
                idx[:, PCOLS:], ga[:, PCOLS : PCOLS + SCOLS], ub[:, PCOLS:], add
            )

            w2v = blob[:, OFF_W2 : OFF_W2 + 1024].bitcast(bf16)   # [128, 512]
            w3v = blob[:, OFF_W3 : OFF_W3 + 1024].bitcast(bf16)   # [128, 512]
            b1v = blob[:, OFF_B1 : OFF_B1 + 8].bitcast(f32)       # [128, 2]
            b2v = blob[:, OFF_B2 : OFF_B2 + 8].bitcast(f32)       # [128, 2]
            bmask_sb = blob[:, OFF_BM : OFF_BM + 16].bitcast(bf16)  # [128, 8]
            b3v = blob[:, OFF_B3 : OFF_B3 + 512].bitcast(bf16)    # [128, 256]

            # ---- parity masks (DVE; deps: gate_all DMA only) ----
            par_i = const.tile([128, NP], i16, tag="par_i")
            nc.vector.tensor_scalar(
                par_i[:], ga[:, PCOLS + SCOLS :], 1, None, band
            )
            parf = const.tile([128, NP], f32, tag="parf")
            nc.vector.tensor_copy(parf[:], par_i[:])
            om = const.tile([128, NP, BPC], bf16, tag="om")
            for w in range(NP):
                nc.vector.tensor_scalar(
                    om[:, w, :], bmask_sb[:], parf[:, w : w + 1], None, mult
                )

            ones8 = const.tile([1, BPC], bf16, tag="ones8")
            nc.vector.memset(ones8[:], 1.0)

            # ---- num_idxs registers, materialized once ----
            reg_p = nc.gpsimd.to_reg(P_NIDX)
            reg_s = nc.gpsimd.to_reg(S_NIDX)

            # ---- gathers: scheduled queues + fused transposed reduce ----
            # Cold first-calls cost ~1.5-1.7us each with dispatch depth ~2, so
            # each queue warms up on a cheap SINGLE call, the DMA-heavy PAIR
            # calls go right after the warm-up (maximum drain window), and the
            # load stays uniform at 6 calls (1s + 2p + 3s) per queue.
            gp_tiles = []
            for w in range(NP):
                g = gat.tile([128, 1, 2 * HID], bf16, tag=f"gpair{w}")
                gp_tiles.append(g)
            gs_tiles = []
            for u in range(NS):
                g = gat.tile([128, 1, HID], bf16, tag=f"gsin{u}")
                gs_tiles.append(g)

            def ev(w):
                return gp_tiles[w][:, 0, 0:HID]

            def od(w):
                return gp_tiles[w][:, 0, HID : 2 * HID]

            # Phase cost ~= cold start + ~2.2us per 4-call wave + drain tail,
            # so fewer waves wins; 5-wave 3p+2s balances desc waves against the
            # 448 KB/queue drain. Pairs FIRST: the heavy 128 KB drains start
            # early and the phase ends on cheap 32 KB single drains.
            QSCHED = [
                [("p", q), ("p", 4 + q), ("p", 8 + q), ("s", q), ("s", 4 + q)]
                for q in range(NQ)
            ]
            d = work.tile([128, NP, HID], bf16, tag="d")
            lv = work.tile([128, NP // 2, HID], bf16, tag="lv")
            # hT accumulation: two PSUM groups, one per 128-wide hid chunk
            psum_hT0 = psum.tile([128, BPC], f32, tag="hT0")
            psum_hT1 = psum.tile([128, BPC], f32, tag="hT1")
            psum_hT = [psum_hT0, psum_hT1]
            first_mm = [True, True]

            def fold(lhsT_by_chunk, rhs, stop=False):
                for c in range(2):
                    nc.tensor.matmul(
                        psum_hT[c][:],
                        lhsT_by_chunk(c),
                        rhs,
                        start=first_mm[c],
                        stop=stop,
                    )
                    first_mm[c] = False

            pair_seen = set()
            for pos in range(5):
                for q in range(NQ):
                    kind, w = QSCHED[q][pos]
                    if kind == "p":
                        win = w1_d[w * PWIN_ROWS : (w + 1) * PWIN_ROWS, :].rearrange(
                            "(a two) n -> a (two n)", two=2
                        )
                        nc.gpsimd.dma_gather(
                            gp_tiles[w][:],
                            win,
                            idx[:, w * P_IDXC : (w + 1) * P_IDXC],
                            P_NIDX,
                            reg_p,
                            2 * HID,
                            queue_num=q,
                        )
                        nc.vector.tensor_tensor(d[:, w, :], od(w), ev(w), sub)
                        fold(
                            lambda c, w=w: d[:, w, c * 128 : (c + 1) * 128],
                            om[:, w, :],
                        )
                        pair_seen.add(w)
                        if (w ^ 1) in pair_seen:
                            lo = min(w, w ^ 1)
                            nc.vector.tensor_tensor(
                                lv[:, lo // 2, :], ev(lo), ev(lo + 1), add
                            )
                    else:
                        base = SBASE_POS * V + w * SWIN_ROWS
                        nc.gpsimd.dma_gather(
                            gs_tiles[w][:],
                            w1_d[base : base + SWIN_ROWS, :],
                            idx[:, PCOLS + w * S_IDXC : PCOLS + (w + 1) * S_IDXC],
                            S_NIDX,
                            reg_s,
                            HID,
                            queue_num=q,
                        )
                        fold(
                            lambda c, w=w: gs_tiles[w][
                                0:64, 0, c * 128 : (c + 1) * 128
                            ],
                            bmask_sb[0:64, :],
                        )
            # progressive even-sum chain (each add gated only by its leaves)
            acc = work.tile([128, HID], bf16, tag="acc")
            nc.vector.tensor_tensor(acc[:], lv[:, 0, :], lv[:, 1, :], add)
            for j in range(2, NP // 2):
                nc.vector.tensor_tensor(acc[:], acc[:], lv[:, j, :], add)
            # even-sum contribution closes both PSUM groups
            fold(lambda c: acc[:, c * 128 : (c + 1) * 128], bmask_sb[:], stop=True)

            # ---- tail MLP on transposed [hid, 8] shard ----
            # relu(x + b) on two engines in parallel: c0 via scalar.activation,
            # c1 via DVE dual-op tensor_scalar (add bias, then max 0).
            maxop = mybir.AluOpType.max

            def relu_bias(out_ap, psum_ap, bias_ap, c):
                if c == 0:
                    nc.scalar.activation(out_ap, psum_ap, Relu, bias=bias_ap)
                else:
                    nc.vector.tensor_scalar(
                        out_ap, psum_ap, bias_ap, 0.0, add, maxop
                    )

            hTr = work.tile([128, 2, BPC], bf16, tag="hTr")
            for c in range(2):
                relu_bias(hTr[:, c, :], psum_hT[c][:], b1v[:, c : c + 1], c)
            h2Tr = work.tile([128, 2, BPC], bf16, tag="h2Tr")
            p_h2T0 = psum.tile([128, BPC], f32, tag="p_h2T0")
            p_h2T1 = psum.tile([128, BPC], f32, tag="p_h2T1")
            for c, p_h2T in enumerate([p_h2T0, p_h2T1]):
                nc.tensor.matmul(
                    p_h2T[:],
                    w2v[:, c * 128 : c * 128 + 128],
                    hTr[:, 0, :],
                    start=True,
                    stop=False,
                )
                nc.tensor.matmul(
                    p_h2T[:],
                    w2v[:, 256 + c * 128 : 256 + c * 128 + 128],
                    hTr[:, 1, :],
                    start=False,
                    stop=True,
                )
                relu_bias(h2Tr[:, c, :], p_h2T[:], b2v[:, c : c + 1], c)
            # Final layer in TWO 128-col PSUM groups: group 0 closes ~0.5us
            # before group 1, so its PSUM->SBUF copy overlaps group 1's
            # matmuls; ONE output DMA (two DMAs regressed: ~0.7us fixed each).
            # Bias matmuls first (no data deps, run early off the chain).
            out_sb = work.tile([BPC, OUT], f32, tag="out_sb")
            p_o0 = psum.tile([BPC, 128], f32, tag="p_o0")
            p_o1 = psum.tile([BPC, 128], f32, tag="p_o1")
            for n, p_o in enumerate([p_o0, p_o1]):
                nc.tensor.matmul(
                    p_o[:], ones8[:], b3v[0:1, n * 128 : (n + 1) * 128],
                    start=True, stop=False,
                )
                nc.tensor.matmul(
                    p_o[:], h2Tr[:, 0, :], w3v[:, n * 128 : (n + 1) * 128],
                    start=False, stop=False,
                )
                nc.tensor.matmul(
                    p_o[:], h2Tr[:, 1, :], w3v[:, 256 + n * 128 : 256 + (n + 1) * 128],
                    start=False, stop=True,
                )
                nc.vector.tensor_copy(out_sb[:, n * 128 : (n + 1) * 128], p_o[:])
            nc.sync.dma_start(out_d[:], out_sb[:])

    # NOTE: hoisting the library load above the framework all-engine barrier
    # was tried and REGRESSED ~11us: the barrier's Pool DRAIN waits for all
    # outstanding Pool DMAs, so it absorbed the whole ~10us IRAM fetch and
    # stalled every engine. Keep the load as the first post-barrier Pool op.
    del mpc_inst

    nc.compile()
    return nc


def get_nc():
    if "nc" not in _CACHE:
        _CACHE["nc"] = _build_nc()
    return _CACHE["nc"]


def make_in_maps(gate_seq, W1, b1, W2, b2, W3, b3):
    """Shard/marshal the full inputs into per-core input maps (values untouched:
    pure slicing, transposition, retyping and tiling)."""
    gate_seq = np.asarray(gate_seq)
    import ml_dtypes

    W1 = np.ascontiguousarray(np.asarray(W1).astype(ml_dtypes.bfloat16))
    W2 = np.asarray(W2, dtype=np.float32)
    W3 = np.asarray(W3, dtype=np.float32)
    b1 = np.asarray(b1, dtype=np.float32)
    b2 = np.asarray(b2, dtype=np.float32)
    b3 = np.asarray(b3, dtype=np.float32)

    # W2 chunked for lhsT use: w2lh[p, kc, nc, f] = W2[kc*128 + p, nc*128 + f]
    # packed flat as [128, (kc nc f)] -> cols k*256 + c*128 + f
    w2lh = np.ascontiguousarray(
        W2.reshape(2, 128, 2, 128).transpose(1, 0, 2, 3).astype(ml_dtypes.bfloat16)
    ).reshape(128, 512)
    # W3 as [128, (k n)]: w3r[p, k*256 + n] = W3[k*128 + p, n]
    w3r = np.ascontiguousarray(
        W3.reshape(2, 128, 256).transpose(1, 0, 2).astype(ml_dtypes.bfloat16)
    ).reshape(128, 512)
    b1t = np.ascontiguousarray(b1.reshape(2, 128).T)  # b1t[p, c] = b1[c*128 + p]
    b2t = np.ascontiguousarray(b2.reshape(2, 128).T)
    bmask = (
        np.arange(128)[:, None] % BPC == np.arange(BPC)[None, :]
    ).astype(ml_dtypes.bfloat16)
    b3rep = np.broadcast_to(b3[None, :], (128, OUT)).astype(ml_dtypes.bfloat16)
    ubias = _host_consts()

    const_part = np.concatenate(
        [
            np.ascontiguousarray(ubias).view(np.uint8),
            w2lh.view(np.uint8),
            w3r.view(np.uint8),
            b1t.astype(np.float32).view(np.uint8),
            b2t.astype(np.float32).view(np.uint8),
            np.ascontiguousarray(bmask).view(np.uint8),
            np.ascontiguousarray(b3rep).view(np.uint8),
        ],
        axis=1,
    )

    # index-layout permutations (see module docstring)
    p16 = np.arange(16)[:, None]
    # pair half
    colp = np.arange(PCOLS)[None, :]
    ip = (colp % P_IDXC) * 16 + p16
    bp = ip % BPC
    tp = (colp // P_IDXC) * PWIN_POS + ip // BPC
    # single half
    cols = np.arange(SCOLS)[None, :]
    i_s = (cols % S_IDXC) * 16 + p16
    bs = i_s % BPC
    ts = SBASE_POS + (cols // S_IDXC) * SWIN_POS + i_s // BPC
    b_idx = np.concatenate([bp, bs], axis=1)
    t_idx = np.concatenate(
        [np.broadcast_to(tp, bp.shape), np.broadcast_to(ts, bs.shape)], axis=1
    )
    # gate_T[p, w] = gate_seq[8m + p%8, w*16 + p//8]  (pair windows only)
    pp = np.arange(128)[:, None]
    ww = np.arange(NP)[None, :]
    bt_idx = np.broadcast_to(pp % BPC, (128, NP))
    tt_idx = ww * PWIN_POS + pp // BPC

    in_maps = []
    for m in range(NCORES):
        gs = gate_seq[m * BPC : (m + 1) * BPC, :]    # [8, 256]
        A = gs[b_idx, t_idx].astype(np.int16)        # [16, PCOLS+SCOLS]
        gate_prep = np.tile(A, (8, 1))               # [128, PCOLS+SCOLS]
        gate_t = gs[bt_idx, tt_idx].astype(np.int16)  # [128, NP]
        gate_all = np.concatenate([gate_prep, gate_t], axis=1)
        blob = np.ascontiguousarray(
            np.concatenate(
                [np.ascontiguousarray(gate_all).view(np.uint8), const_part],
                axis=1,
            )
        )
        assert blob.shape == (128, BLOB_BYTES), blob.shape
        in_maps.append({"w1": W1, "blob": blob})
    return in_maps


def run(inputs, trace=False, **spmd_kwargs):
    from concourse.bass_utils import run_bass_kernel_spmd

    nc = get_nc()
    in_maps = make_in_maps(**inputs)
    res = run_bass_kernel_spmd(
        nc, in_maps, core_ids=list(range(NCORES)), trace=trace, **spmd_kwargs
    )
    out = np.concatenate([r["out"] for r in res.results], axis=0)
    return out, res


def kernel(**inputs) -> np.ndarray:
    out, _ = run(inputs, trace=False)
    return out
